# revision 31
# baseline (speedup 1.0000x reference)
"""Binarized linear + BatchNorm (eval) on 8 Trainium2 NeuronCores.

Computes: out = BN(sign(x) @ sign(weight).T)  for
  x [8192, 4096] f32, weight [4096, 4096] f32, BN vectors [4096] f32.

Strategy
--------
Sharding: batch 4-way x out_features 2-way (8 cores). Each core computes
outT [2048(O), 2048(B)] f32 locally; no collectives; the host
concatenates the 8 tiles.

sign(x) and sign(weight) are +/-1 exactly, so they are computed on the
HOST and shipped as fp8e4 bytes (value-exact; 1.0 = 0x38 in e4m3).
This halves input DMA vs bf16 and removes every on-device activation:
the device program is nothing but DMA-in -> fp8 DoubleRow matmuls
(K=256/instr, 2x bf16 rate, exact in fp32 PSUM) -> a fused
out = a*acc + b BN drain on the vector engine -> DMA-out.

Both operands are laid out host-side as the exact SBUF image
([128 partitions, contig]), so every input DMA is a plain 2D
[128 x multi-KB-contiguous] descriptor at full HBM efficiency, chunked
only as far as needed for the PE to chase the stream: X batch-tile 0 in
4 chunks, the rest coarser; W out-tile 0 first in halves so the first
matmul can issue ~2us in. BN constants a = gamma/sqrt(var+eps),
b = beta - mean*a are precomputed on host ([128, 2*OT] image).
"""

import numpy as np
from contextlib import ExitStack

B_FULL, IN, OUT = 8192, 4096, 4096
NB_CORES = 8
BI, OI = 4, 2            # batch x out_features core grid
BS = B_FULL // BI        # 2048 batch per core
OS = OUT // OI           # 2048 out_features per core
KT = IN // 128           # 32 k-tiles of 128
NS = KT // 2             # 16 k256 supertiles (DoubleRow)
OT = OS // 128           # 16 out tiles of 128
NBT = BS // 512          # 4 batch tiles of 512
BN_EPS = 1e-05

_CACHE = {}


def _build_program():
    import concourse.tile as tile
    from concourse import mybir, bacc

    F8 = mybir.dt.float8e4
    F16 = mybir.dt.float16
    F32 = mybir.dt.float32
    DR = mybir.MatmulPerfMode.DoubleRow

    nc = bacc.Bacc("TRN2", target_bir_lowering=False, debug=False,
                   num_devices=NB_CORES)
    # SBUF-image layouts: [128, NBT*KT, 512] / [128, OT*KT, 128], so every
    # DMA below is [128 partitions x contiguous bytes] on both sides.
    xq = nc.declare_dram_parameter("xq", [128, NBT * KT, 512], F8, isOutput=False)
    wq = nc.declare_dram_parameter("wq", [128, OT * KT, 128], F8, isOutput=False)
    abv = nc.declare_dram_parameter("abv", [128, 2 * OT], F32, isOutput=False)
    o = nc.declare_dram_parameter("o", [OS, BS], F16, isOutput=True)

    with tile.TileContext(nc) as tc:
        with ExitStack() as ctx:
            cons = ctx.enter_context(tc.tile_pool(name="cons", bufs=1))
            obp = ctx.enter_context(tc.tile_pool(name="ob", bufs=4))
            psp = ctx.enter_context(tc.tile_pool(name="ps", bufs=5, space="PSUM"))

            ab_sb = cons.tile([128, 2 * OT], F32)
            xb = cons.tile([128, NBT * KT, 512], F8)   # 64KB/partition
            wb = cons.tile([128, OT * KT, 128], F8)    # 64KB/partition
            dum = cons.tile([128, 2, 640], F8)         # warm-matmul feed

            # DMA completion semaphores are a small global pool reused
            # round-robin in program order (a DMA's issue waits for the
            # one ~9 slots back to finish), so: ab first (tiny, frees its
            # sem immediately), every input DMA before any output DMA,
            # and X/W interleaved so the first ~9 are the urgent ones.
            nc.gpsimd.dma_start(ab_sb[:], abv[:])
            # X nb0's last supertiles ride the otherwise-idle SWDGE ring:
            # even at its ~40-70 GB/s, 256KB lands by ~16us, so the sync
            # ring finishes nb0 ~1.2us earlier (its stream end bounds the
            # whole schedule: last-mm-end = nb0-end + 63 blocks).
            nc.gpsimd.dma_start(xb[:, 28:KT, :], xq[:, 28:KT, :])

            def w_dma(k0, k1):
                nc.scalar.dma_start(wb[:, k0:k1, :], wq[:, k0:k1, :])

            def x_dma(nb, k0, k1):
                nc.sync.dma_start(xb[:, nb * KT + k0:nb * KT + k1, :],
                                  xq[:, nb * KT + k0:nb * KT + k1, :])

            # Only 8 HWDGE completion-sem lanes exist globally; a 9th
            # outstanding DMA's issue waits for a completion, so early
            # chunks must stay few and coarse or the rings starve.
            # Block 0 is gated by X nb0's ~9us stream (one ring), so what
            # matters is when its LAST bytes land: front-load big chunks,
            # ship the tail small on the scalar ring (which is otherwise
            # well ahead after W ot0/ot1). Matmul waits are coalesced by
            # the scheduler, so fine chunking below ~6 k-tiles is wasted.
            # The scalar (W) ring sustains only ~175 GB/s -- just-in-time
            # against the 145 GB/s demand of wave 0, and matmul waits are
            # hoisted ~8 instructions early, so each W tile must land
            # ~1.7us before its block. Relieve the pinch at ot2/ot3 by
            # carrying ot2 on the sync ring (idle after X nb0) and
            # shifting the pair stream one slot earlier on scalar.
            x_dma(0, 0, 4)
            w_dma(0, 8)
            x_dma(0, 4, 8)
            w_dma(8, KT)
            x_dma(0, 8, 16)
            w_dma(KT, 2 * KT)
            x_dma(0, 16, 24)
            w_dma(3 * KT, 4 * KT)
            x_dma(0, 24, 28)
            w_dma(4 * KT, 6 * KT)
            nc.sync.dma_start(wb[:, 2 * KT:3 * KT, :], wq[:, 2 * KT:3 * KT, :])
            w_dma(6 * KT, 8 * KT)
            x_dma(1, 0, KT)
            w_dma(8 * KT, 10 * KT)
            x_dma(2, 0, KT)
            w_dma(10 * KT, 12 * KT)
            x_dma(3, 0, KT)
            w_dma(12 * KT, 14 * KT)
            w_dma(14 * KT, 16 * KT)

            # Warm matmuls on dummy data keep the PE streaming through
            # the input-gated windows (pre-block-0 idle and block-0's
            # stream stalls). The clock governor throttles the PE after
            # idle gaps >~3.4us -- a throttled run costs ~45us -- and
            # these fills never delay a real matmul: they only occupy
            # cycles the PE would have spent waiting on DMA semaphores.
            warm = psp.tile([128, 512], F32, tag="warm", bufs=1, name="warm")
            nc.vector.memset(dum[:], 1.0)

            def warm_mm(n):
                for _ in range(n):
                    nc.tensor.matmul(warm[:], dum[:, :, 0:128],
                                     dum[:, :, 128:640],
                                     start=True, stop=True, perf_mode=DR)

            warm_mm(12)

            def do_mm(acc, ot, nb, s):
                nc.tensor.matmul(
                    acc[:],
                    wb[:, ot * KT + 2 * s:ot * KT + 2 * s + 2, :],
                    xb[:, nb * KT + 2 * s:nb * KT + 2 * s + 2, :],
                    start=(s == 0), stop=(s == NS - 1),
                    perf_mode=DR)

            def bn_out(acc, ot, nb):
                # fp16 output: the out ring moves ~73 GB/s, f32 output
                # is produced at ~74 GB/s -- fp16 halves it so the ring
                # never backpressures the PE. |out| <= ~1400 so fp16
                # rounding is <= 0.5 absolute, ~4e-4 relative.
                blk = nb * OT + ot
                if blk == NBT * OT - 1:
                    # final block: split BN + output across both HWDGE
                    # rings so the end-of-kernel drain starts ~1us
                    # earlier (this BN->DMA chain is the tail's
                    # critical path into the fixed ~9us epilogue).
                    for h, ring in ((0, nc.sync), (1, nc.scalar)):
                        obh = obp.tile([128, 256], F16, tag=f"obh{h}",
                                       bufs=1, name=f"obh{h}")
                        nc.vector.tensor_scalar(
                            obh[:], acc[:, h * 256:(h + 1) * 256],
                            ab_sb[:, ot:ot + 1],
                            ab_sb[:, OT + ot:OT + ot + 1],
                            mybir.AluOpType.mult, mybir.AluOpType.add)
                        ring.dma_start(
                            o[ot * 128:(ot + 1) * 128,
                              nb * 512 + h * 256:nb * 512 + (h + 1) * 256],
                            obh[:])
                    return
                ob = obp.tile([128, 512], F16, tag="ob",
                              name=f"ob_{ot}_{nb}")
                nc.vector.tensor_scalar(
                    ob[:], acc[:],
                    ab_sb[:, ot:ot + 1], ab_sb[:, OT + ot:OT + ot + 1],
                    mybir.AluOpType.mult, mybir.AluOpType.add)
                # last blocks' outputs ride the (long-idle) HWDGE rings
                # so the SWDGE ring is empty well before kernel end --
                # its end-of-kernel DRAIN otherwise burns ~6us.
                if blk < NBT * OT - 8:
                    oring = nc.gpsimd
                else:
                    oring = nc.sync if blk % 2 == 0 else nc.scalar
                oring.dma_start(
                    o[ot * 128:(ot + 1) * 128, nb * 512:(nb + 1) * 512],
                    ob[:])

            # Wave-0 pace from block 2 on is W-supply-bound (one 512KB
            # out-tile per ~2.6us from the scalar ring vs 3.46us/block
            # demand with little cushion), so finishing blocks 0/1 early
            # buys nothing -- block 0 simply chases the X stream with
            # warm fills in the gaps.
            for nb in range(NBT):
                for ot in range(OT):
                    acc = psp.tile([128, 512], F32, tag="acc",
                                   name=f"acc_{ot}_{nb}")
                    for s in range(NS):
                        do_mm(acc, ot, nb, s)
                        if nb == 0 and ot == 0:
                            warm_mm(10 if s == 3 else (1 if s < 14 else 0))
                    bn_out(acc, ot, nb)

    nc.compile()
    return nc


def make_in_maps(x, weight, bn_gamma, bn_beta, bn_mean, bn_var):
    import ml_dtypes
    f8 = getattr(ml_dtypes, "float8_e4m3", None) or ml_dtypes.float8_e4m3fn

    # X shards: [128, NBT*KT, 512] with (p, nb*KT+kt, b) = sign(x)[nb*512+b,
    # kt*128+p] -- the SBUF image the kernel DMAs verbatim.
    xqs = []
    for bi in range(BI):
        xs8 = np.sign(x[bi * BS:(bi + 1) * BS, :]).astype(f8)
        t = xs8.reshape(NBT, 512, KT, 128).transpose(3, 0, 2, 1)
        xqs.append(np.ascontiguousarray(t.reshape(128, NBT * KT, 512)))
    # W shards: [128, OT*KT, 128] with (p, ot*KT+kt, q) = sign(w)[ot*128+q,
    # kt*128+p].
    wqs = []
    abs_ = []
    std = np.sqrt(bn_var + np.float32(BN_EPS))
    a_full = bn_gamma / std
    b_full = bn_beta - bn_mean * a_full
    for oi in range(OI):
        sl = slice(oi * OS, (oi + 1) * OS)
        ws8 = np.sign(weight[sl, :]).astype(f8)
        t = ws8.reshape(OT, 128, KT, 128).transpose(3, 0, 2, 1)
        wqs.append(np.ascontiguousarray(t.reshape(128, OT * KT, 128)))
        abs_.append(np.ascontiguousarray(np.concatenate(
            [a_full[sl].reshape(OT, 128).T, b_full[sl].reshape(OT, 128).T],
            axis=1, dtype=np.float32)))
    in_maps = []
    for c in range(NB_CORES):
        bi, oi = divmod(c, OI)
        in_maps.append({"xq": xqs[bi], "wq": wqs[oi], "abv": abs_[oi]})
    return in_maps


def kernel(x, weight, bn_gamma, bn_beta, bn_mean, bn_var):
    from concourse.bass_utils import run_bass_kernel_spmd

    x = np.asarray(x, dtype=np.float32)
    weight = np.asarray(weight, dtype=np.float32)
    bn_gamma = np.asarray(bn_gamma, dtype=np.float32)
    bn_beta = np.asarray(bn_beta, dtype=np.float32)
    bn_mean = np.asarray(bn_mean, dtype=np.float32)
    bn_var = np.asarray(bn_var, dtype=np.float32)

    if "nc" not in _CACHE:
        _CACHE["nc"] = _build_program()
    nc = _CACHE["nc"]

    in_maps = make_in_maps(x, weight, bn_gamma, bn_beta, bn_mean, bn_var)

    res = run_bass_kernel_spmd(nc, in_maps, list(range(NB_CORES)))
    _CACHE["last_results"] = res

    out = np.empty((B_FULL, OUT), dtype=np.float32)
    for c in range(NB_CORES):
        bi, oi = divmod(c, OI)
        out[bi * BS:(bi + 1) * BS, oi * OS:(oi + 1) * OS] = \
            res.results[c]["o"].T.astype(np.float32)
    return out


# revision 33
# speedup vs baseline: 1.0148x; 1.0148x over previous
"""Binarized linear + BatchNorm (eval) on 8 Trainium2 NeuronCores.

Computes: out = BN(sign(x) @ sign(weight).T)  for
  x [8192, 4096] f32, weight [4096, 4096] f32, BN vectors [4096] f32.

Strategy
--------
Sharding: batch 4-way x out_features 2-way (8 cores). Each core computes
outT [2048(O), 2048(B)] f32 locally; no collectives; the host
concatenates the 8 tiles.

sign(x) and sign(weight) are +/-1 exactly, so they are computed on the
HOST and shipped as fp8e4 bytes (value-exact; 1.0 = 0x38 in e4m3).
This halves input DMA vs bf16 and removes every on-device activation:
the device program is nothing but DMA-in -> fp8 DoubleRow matmuls
(K=256/instr, 2x bf16 rate, exact in fp32 PSUM) -> a fused
out = a*acc + b BN drain on the vector engine -> DMA-out.

Both operands are laid out host-side as the exact SBUF image
([128 partitions, contig]), so every input DMA is a plain 2D
[128 x multi-KB-contiguous] descriptor at full HBM efficiency, chunked
only as far as needed for the PE to chase the stream: X batch-tile 0 in
4 chunks, the rest coarser; W out-tile 0 first in halves so the first
matmul can issue ~2us in. BN constants a = gamma/sqrt(var+eps),
b = beta - mean*a are precomputed on host ([128, 2*OT] image).
"""

import numpy as np
from contextlib import ExitStack

B_FULL, IN, OUT = 8192, 4096, 4096
NB_CORES = 8
BI, OI = 4, 2            # batch x out_features core grid
BS = B_FULL // BI        # 2048 batch per core
OS = OUT // OI           # 2048 out_features per core
KT = IN // 128           # 32 k-tiles of 128
NS = KT // 2             # 16 k256 supertiles (DoubleRow)
OT = OS // 128           # 16 out tiles of 128
NBT = BS // 512          # 4 batch tiles of 512
BN_EPS = 1e-05

_CACHE = {}


def _build_program():
    import concourse.tile as tile
    from concourse import mybir, bacc

    F8 = mybir.dt.float8e4
    F16 = mybir.dt.float16
    F32 = mybir.dt.float32
    DR = mybir.MatmulPerfMode.DoubleRow

    nc = bacc.Bacc("TRN2", target_bir_lowering=False, debug=False,
                   num_devices=NB_CORES)
    # SBUF-image layouts: [128, NBT*KT, 512] / [128, OT*KT, 128], so every
    # DMA below is [128 partitions x contiguous bytes] on both sides.
    xq = nc.declare_dram_parameter("xq", [128, NBT * KT, 512], F8, isOutput=False)
    wq = nc.declare_dram_parameter("wq", [128, OT * KT, 128], F8, isOutput=False)
    abv = nc.declare_dram_parameter("abv", [128, 2 * OT], F32, isOutput=False)
    o = nc.declare_dram_parameter("o", [OS, BS], F16, isOutput=True)

    with tile.TileContext(nc) as tc:
        with ExitStack() as ctx:
            cons = ctx.enter_context(tc.tile_pool(name="cons", bufs=1))
            obp = ctx.enter_context(tc.tile_pool(name="ob", bufs=4))
            psp = ctx.enter_context(tc.tile_pool(name="ps", bufs=5, space="PSUM"))

            ab_sb = cons.tile([128, 2 * OT], F32)
            xb = cons.tile([128, NBT * KT, 512], F8)   # 64KB/partition
            wb = cons.tile([128, OT * KT, 128], F8)    # 64KB/partition
            dum = cons.tile([128, 2, 640], F8)         # warm-matmul feed

            # DMA completion semaphores are a small global pool reused
            # round-robin in program order (a DMA's issue waits for the
            # one ~9 slots back to finish), so: ab first (tiny, frees its
            # sem immediately), every input DMA before any output DMA,
            # and X/W interleaved so the first ~9 are the urgent ones.
            nc.gpsimd.dma_start(ab_sb[:], abv[:])
            # X nb0's last supertiles ride the otherwise-idle SWDGE ring:
            # even at its ~40-70 GB/s, 256KB lands by ~16us, so the sync
            # ring finishes nb0 ~1.2us earlier (its stream end bounds the
            # whole schedule: last-mm-end = nb0-end + 63 blocks).
            nc.gpsimd.dma_start(xb[:, 28:KT, :], xq[:, 28:KT, :])

            def w_dma(k0, k1):
                nc.scalar.dma_start(wb[:, k0:k1, :], wq[:, k0:k1, :])

            def x_dma(nb, k0, k1):
                nc.sync.dma_start(xb[:, nb * KT + k0:nb * KT + k1, :],
                                  xq[:, nb * KT + k0:nb * KT + k1, :])

            # Only 8 HWDGE completion-sem lanes exist globally; a 9th
            # outstanding DMA's issue waits for a completion, so early
            # chunks must stay few and coarse or the rings starve.
            # Block 0 is gated by X nb0's ~9us stream (one ring), so what
            # matters is when its LAST bytes land: front-load big chunks,
            # ship the tail small on the scalar ring (which is otherwise
            # well ahead after W ot0/ot1). Matmul waits are coalesced by
            # the scheduler, so fine chunking below ~6 k-tiles is wasted.
            # The scalar (W) ring sustains only ~175 GB/s -- just-in-time
            # against the 145 GB/s demand of wave 0, and matmul waits are
            # hoisted ~8 instructions early, so each W tile must land
            # ~1.7us before its block. Relieve the pinch at ot2/ot3 by
            # carrying ot2 on the sync ring (idle after X nb0) and
            # shifting the pair stream one slot earlier on scalar.
            x_dma(0, 0, 4)
            w_dma(0, 8)
            x_dma(0, 4, 8)
            w_dma(8, KT)
            x_dma(0, 8, 16)
            w_dma(KT, 2 * KT)
            x_dma(0, 16, 24)
            w_dma(3 * KT, 4 * KT)
            x_dma(0, 24, 28)
            w_dma(4 * KT, 6 * KT)
            nc.sync.dma_start(wb[:, 2 * KT:3 * KT, :], wq[:, 2 * KT:3 * KT, :])
            w_dma(6 * KT, 8 * KT)
            x_dma(1, 0, KT)
            w_dma(8 * KT, 10 * KT)
            x_dma(2, 0, KT)
            w_dma(10 * KT, 12 * KT)
            x_dma(3, 0, KT)
            w_dma(12 * KT, 14 * KT)
            w_dma(14 * KT, 16 * KT)

            # Warm matmuls on dummy data keep the PE streaming through
            # the input-gated windows (pre-block-0 idle and block-0's
            # stream stalls). The clock governor throttles the PE after
            # idle gaps >~3.4us -- a throttled run costs ~45us -- and
            # these fills never delay a real matmul: they only occupy
            # cycles the PE would have spent waiting on DMA semaphores.
            warm = psp.tile([128, 512], F32, tag="warm", bufs=1, name="warm")
            nc.vector.memset(dum[:], 1.0)

            def warm_mm(n):
                for _ in range(n):
                    nc.tensor.matmul(warm[:], dum[:, :, 0:128],
                                     dum[:, :, 128:640],
                                     start=True, stop=True, perf_mode=DR)

            warm_mm(12)

            def do_mm(acc, ot, nb, s):
                nc.tensor.matmul(
                    acc[:],
                    wb[:, ot * KT + 2 * s:ot * KT + 2 * s + 2, :],
                    xb[:, nb * KT + 2 * s:nb * KT + 2 * s + 2, :],
                    start=(s == 0), stop=(s == NS - 1),
                    perf_mode=DR)

            def bn_out(acc, ot, nb):
                # fp16 output: the out ring moves ~73 GB/s, f32 output
                # is produced at ~74 GB/s -- fp16 halves it so the ring
                # never backpressures the PE. |out| <= ~1400 so fp16
                # rounding is <= 0.5 absolute, ~4e-4 relative.
                blk = nb * OT + ot
                if blk == NBT * OT - 1:
                    # final block: split BN + output across both HWDGE
                    # rings so the end-of-kernel drain starts ~1us
                    # earlier (this BN->DMA chain is the tail's
                    # critical path into the fixed ~9us epilogue).
                    for h, ring in ((0, nc.sync), (1, nc.scalar)):
                        obh = obp.tile([128, 256], F16, tag=f"obh{h}",
                                       bufs=1, name=f"obh{h}")
                        nc.vector.tensor_scalar(
                            obh[:], acc[:, h * 256:(h + 1) * 256],
                            ab_sb[:, ot:ot + 1],
                            ab_sb[:, OT + ot:OT + ot + 1],
                            mybir.AluOpType.mult, mybir.AluOpType.add)
                        ring.dma_start(
                            o[ot * 128:(ot + 1) * 128,
                              nb * 512 + h * 256:nb * 512 + (h + 1) * 256],
                            obh[:])
                    return
                ob = obp.tile([128, 512], F16, tag="ob",
                              name=f"ob_{ot}_{nb}")
                nc.vector.tensor_scalar(
                    ob[:], acc[:],
                    ab_sb[:, ot:ot + 1], ab_sb[:, OT + ot:OT + ot + 1],
                    mybir.AluOpType.mult, mybir.AluOpType.add)
                # last blocks' outputs ride the (long-idle) HWDGE rings
                # so the SWDGE ring is empty well before kernel end --
                # its end-of-kernel DRAIN otherwise burns ~6us.
                if blk < NBT * OT - 8:
                    oring = nc.gpsimd
                else:
                    oring = nc.sync if blk % 2 == 0 else nc.scalar
                oring.dma_start(
                    o[ot * 128:(ot + 1) * 128, nb * 512:(nb + 1) * 512],
                    ob[:])

            # Wave-0 pace from block 2 on is W-supply-bound (one 512KB
            # out-tile per ~2.6us from the scalar ring vs 3.46us/block
            # demand with little cushion), so finishing blocks 0/1 early
            # buys nothing -- block 0 simply chases the X stream with
            # warm fills in the gaps.
            for nb in range(NBT):
                for ot in range(OT):
                    acc = psp.tile([128, 512], F32, tag="acc",
                                   name=f"acc_{ot}_{nb}")
                    for s in range(NS):
                        do_mm(acc, ot, nb, s)
                        if nb == 0 and ot == 0:
                            warm_mm(10 if s == 3 else (1 if s < 14 else 0))
                    bn_out(acc, ot, nb)

    nc.compile()
    return nc


def make_in_maps(x, weight, bn_gamma, bn_beta, bn_mean, bn_var):
    import ml_dtypes
    f8 = getattr(ml_dtypes, "float8_e4m3", None) or ml_dtypes.float8_e4m3fn

    # X shards: [128, NBT*KT, 512] with (p, nb*KT+kt, b) = sign(x)[nb*512+b,
    # kt*128+p] -- the SBUF image the kernel DMAs verbatim.
    xqs = []
    for bi in range(BI):
        xs8 = np.sign(x[bi * BS:(bi + 1) * BS, :]).astype(f8)
        t = xs8.reshape(NBT, 512, KT, 128).transpose(3, 0, 2, 1)
        xqs.append(np.ascontiguousarray(t.reshape(128, NBT * KT, 512)))
    # W shards: [128, OT*KT, 128] with (p, ot*KT+kt, q) = sign(w)[ot*128+q,
    # kt*128+p].
    wqs = []
    abs_ = []
    std = np.sqrt(bn_var + np.float32(BN_EPS))
    a_full = bn_gamma / std
    b_full = bn_beta - bn_mean * a_full
    for oi in range(OI):
        sl = slice(oi * OS, (oi + 1) * OS)
        ws8 = np.sign(weight[sl, :]).astype(f8)
        t = ws8.reshape(OT, 128, KT, 128).transpose(3, 0, 2, 1)
        wqs.append(np.ascontiguousarray(t.reshape(128, OT * KT, 128)))
        abs_.append(np.ascontiguousarray(np.concatenate(
            [a_full[sl].reshape(OT, 128).T, b_full[sl].reshape(OT, 128).T],
            axis=1, dtype=np.float32)))
    in_maps = []
    for c in range(NB_CORES):
        bi, oi = divmod(c, OI)
        in_maps.append({"xq": xqs[bi], "wq": wqs[oi], "abv": abs_[oi]})
    return in_maps


def kernel(x, weight, bn_gamma, bn_beta, bn_mean, bn_var):
    from concourse.bass_utils import run_bass_kernel_spmd

    x = np.asarray(x, dtype=np.float32)
    weight = np.asarray(weight, dtype=np.float32)
    bn_gamma = np.asarray(bn_gamma, dtype=np.float32)
    bn_beta = np.asarray(bn_beta, dtype=np.float32)
    bn_mean = np.asarray(bn_mean, dtype=np.float32)
    bn_var = np.asarray(bn_var, dtype=np.float32)

    if "nc" not in _CACHE:
        _CACHE["nc"] = _build_program()
    nc = _CACHE["nc"]

    in_maps = make_in_maps(x, weight, bn_gamma, bn_beta, bn_mean, bn_var)

    res = run_bass_kernel_spmd(nc, in_maps, list(range(NB_CORES)))
    _CACHE["last_results"] = res

    out = np.empty((B_FULL, OUT), dtype=np.float32)
    for c in range(NB_CORES):
        bi, oi = divmod(c, OI)
        out[bi * BS:(bi + 1) * BS, oi * OS:(oi + 1) * OS] = \
            res.results[c]["o"].T.astype(np.float32)
    return out
